# revision 2
# baseline (speedup 1.0000x reference)
"""DiffLogicLayer forward on 8 TRN2 NeuronCores.

Math: every one of the 16 soft logic ops is affine in {1, a, b, a*b}, so
    out[n, o] = C0[o] + C1[o]*a + C2[o]*b + C3[o]*a*b
with a = x[n, conn_a[o]], b = x[n, conn_b[o]] and C = softmax(weights) @ M
for the constant 16x4 matrix M of op coefficients.

Sharding: out_dim (gate axis) split 8 ways; each core owns 1024 gates and
the full batch. The host pre-gathers the two operand rows per gate from
x.T (pure data movement, part of the shard staging) and ships them as
fp16 [1024, 4096] streams aT/bT, so the device does no gathering at all:
per 128-gate slot it streams a and b in with plain HWDGE DMAs, computes
    u = C3*a + C2   (ACT, per-partition scale/bias)
    w = C1*a + C0   (ACT)
    v = u * b       (DVE, fp16 2x mode)
    o = v + w       (DVE)
and DMAs the fp16 [128, 4096] slot to outT. C0..C3 are computed on-device
in f32 from the weights shard (exp -> strided-window reduces -> signed
sums -> * 1/Z). Host transposes/concats/casts the per-core outT shards
into the full f32 output.
"""

import numpy as np
from contextlib import ExitStack

import concourse.bacc as bacc
import concourse.mybir as mybir
import concourse.tile as tile
from concourse.bass_utils import run_bass_kernel_spmd

N_CORES = 8
BATCH, IN_DIM, OUT_DIM = 4096, 4096, 8192
GPC = OUT_DIM // N_CORES          # gates per core = 1024
SLOTS = GPC // 128                # 128-gate slots per core = 8
F32 = mybir.dt.float32
F16 = mybir.dt.float16

_compiled = {}


def _build_nc():
    nc = bacc.Bacc("TRN2", target_bir_lowering=False, debug=False,
                   num_devices=N_CORES)
    aT = nc.dram_tensor("aT", [GPC, BATCH], F16, kind="ExternalInput")
    bT = nc.dram_tensor("bT", [GPC, BATCH], F16, kind="ExternalInput")
    wt = nc.dram_tensor("wt", [GPC, 16], F32, kind="ExternalInput")
    outT = nc.dram_tensor("outT", [GPC, BATCH], F16, kind="ExternalOutput")

    with tile.TileContext(nc) as tc, ExitStack() as ctx:
        const = ctx.enter_context(tc.tile_pool(name="const", bufs=1))
        pa = ctx.enter_context(tc.tile_pool(name="a", bufs=3))
        pb = ctx.enter_context(tc.tile_pool(name="b", bufs=3))
        pu = ctx.enter_context(tc.tile_pool(name="u", bufs=2))
        pw = ctx.enter_context(tc.tile_pool(name="w", bufs=2))
        po = ctx.enter_context(tc.tile_pool(name="o", bufs=2))

        # ---- per-gate coefficients from weights ----
        W = const.tile([128, SLOTS, 16], F32, tag="W")
        nc.sync.dma_start(W[:], wt.ap().rearrange("(s p) i -> p s i", p=128))
        E = const.tile([128, SLOTS, 16], F32, tag="E")
        nc.scalar.activation(E[:], W[:], mybir.ActivationFunctionType.Exp)

        def red(lo, hi, tag):
            t = const.tile([128, SLOTS], F32, tag=tag)
            nc.vector.tensor_reduce(t[:], E[:, :, lo:hi],
                                    mybir.AxisListType.X, mybir.AluOpType.add)
            return t

        Z = red(0, 16, "Z")
        R = const.tile([128, SLOTS], F32, tag="R")
        nc.vector.reciprocal(R[:], Z[:])

        # C0 = e8..e15
        C0 = red(8, 16, "C0")
        # C1 = (e2+e3) + (e6+e7) - (e8+e9) - (e12+e13)
        P23, P67, P89, P1213 = (red(2, 4, "P23"), red(6, 8, "P67"),
                                red(8, 10, "P89"), red(12, 14, "P1213"))
        C1 = const.tile([128, SLOTS], F32, tag="C1")
        nc.vector.tensor_add(C1[:], P23[:], P67[:])
        nc.vector.tensor_sub(C1[:], C1[:], P89[:])
        nc.vector.tensor_sub(C1[:], C1[:], P1213[:])
        # C2 = (e4..e7) - (e8+e9) - (e10+e11)
        P4567, P1011 = red(4, 8, "P4567"), red(10, 12, "P1011")
        C2 = const.tile([128, SLOTS], F32, tag="C2")
        nc.vector.tensor_sub(C2[:], P4567[:], P89[:])
        nc.vector.tensor_sub(C2[:], C2[:], P1011[:])
        # C3 = e1 - e2 - e4 - 2e6 - e7 + e8 + 2e9 + e11 + e13 - e14
        #    = (e1+e8+e11+e13) + 2(e9-e6) - (e2+e4+e7+e14)
        def sl(i):
            return E[:, :, i]

        C3 = const.tile([128, SLOTS], F32, tag="C3")
        t1 = const.tile([128, SLOTS], F32, tag="t1")
        nc.vector.tensor_add(C3[:], sl(1), sl(8))
        nc.vector.tensor_add(C3[:], C3[:], sl(11))
        nc.vector.tensor_add(C3[:], C3[:], sl(13))
        nc.vector.tensor_sub(t1[:], sl(9), sl(6))
        nc.vector.tensor_add(C3[:], C3[:], t1[:])
        nc.vector.tensor_add(C3[:], C3[:], t1[:])
        nc.vector.tensor_add(t1[:], sl(2), sl(4))
        nc.vector.tensor_add(t1[:], t1[:], sl(7))
        nc.vector.tensor_add(t1[:], t1[:], sl(14))
        nc.vector.tensor_sub(C3[:], C3[:], t1[:])
        # normalize by softmax denominator
        for C in (C0, C1, C2, C3):
            nc.vector.tensor_mul(C[:], C[:], R[:])

        # ---- main loop over 128-gate slots ----
        for s in range(SLOTS):
            a = pa.tile([128, BATCH], F16, tag="a")
            nc.sync.dma_start(a[:], aT.ap()[s * 128:(s + 1) * 128, :])
            b = pb.tile([128, BATCH], F16, tag="b")
            nc.sync.dma_start(b[:], bT.ap()[s * 128:(s + 1) * 128, :])
            u = pu.tile([128, BATCH], F16, tag="u")
            nc.scalar.activation(u[:], a[:], mybir.ActivationFunctionType.Identity,
                                 bias=C2[:, s : s + 1], scale=C3[:, s : s + 1])
            w = pw.tile([128, BATCH], F16, tag="w")
            nc.scalar.activation(w[:], a[:], mybir.ActivationFunctionType.Identity,
                                 bias=C0[:, s : s + 1], scale=C1[:, s : s + 1])
            nc.vector.tensor_mul(u[:], u[:], b[:])
            o = po.tile([128, BATCH], F16, tag="o")
            nc.vector.tensor_add(o[:], u[:], w[:])
            nc.sync.dma_start(outT.ap()[s * 128:(s + 1) * 128, :], o[:])

    nc.compile()
    return nc


def make_in_maps(x, weights, conn_a, conn_b):
    x = np.asarray(x, dtype=np.float32)
    weights = np.asarray(weights, dtype=np.float32)
    ca = np.asarray(conn_a).astype(np.int64)
    cb = np.asarray(conn_b).astype(np.int64)
    xT_h = np.ascontiguousarray(x.T).astype(np.float16)
    in_maps = []
    for c in range(N_CORES):
        g0, g1 = c * GPC, (c + 1) * GPC
        in_maps.append({
            "aT": np.ascontiguousarray(xT_h[ca[g0:g1]]),
            "bT": np.ascontiguousarray(xT_h[cb[g0:g1]]),
            "wt": np.ascontiguousarray(weights[g0:g1]),
        })
    return in_maps


def get_nc():
    if "nc" not in _compiled:
        _compiled["nc"] = _build_nc()
    return _compiled["nc"]


def assemble_out(results) -> np.ndarray:
    out = np.empty((BATCH, OUT_DIM), np.float32)
    for c in range(N_CORES):
        out[:, c * GPC:(c + 1) * GPC] = results[c]["outT"].T.astype(np.float32)
    return out


def kernel(x, weights, conn_a, conn_b) -> np.ndarray:
    nc = get_nc()
    in_maps = make_in_maps(x, weights, conn_a, conn_b)
    res = run_bass_kernel_spmd(nc, in_maps, core_ids=list(range(N_CORES)))
    return assemble_out(res.results)


# revision 5
# speedup vs baseline: 14.7881x; 14.7881x over previous
"""DiffLogicLayer forward on 8 TRN2 NeuronCores.

Math: every one of the 16 soft logic ops is affine in {1, a, b, a*b}, so
    out[n, o] = C0[o] + C1[o]*a + C2[o]*b + C3[o]*a*b
with a = x[n, conn_a[o]], b = x[n, conn_b[o]] and C = softmax(weights) @ M
for the constant 16x4 matrix M of op coefficients.

Sharding: out_dim (gate axis) split 8 ways; each core owns 1024 gates and
the full batch. The host pre-gathers the two operand rows per gate from
x.T (pure data movement, part of the shard staging) and ships them as
fp16 [1024, 4096] streams aT/bT, so the device does no gathering at all:
per pair of 128-gate slots it streams a and b in with 2 MiB HWDGE DMAs,
and per slot computes
    u = C3*a + C2   (ACT, per-partition scale/bias)
    w = C1*a + C0   (DVE tensor_scalar, per-partition scalars)
    v = u * b       (DVE, fp16 2x mode)
    o = v + w       (DVE)
and DMAs fp16 [128, 2, 4096] slot pairs to outT. C0..C3 are computed on-device
in f32 from the weights shard (exp -> strided-window reduces -> signed
sums -> * 1/Z). Host transposes/concats/casts the per-core outT shards
into the full f32 output.
"""

import numpy as np
from contextlib import ExitStack

import concourse.bacc as bacc
import concourse.mybir as mybir
import concourse.tile as tile
from concourse.bass_utils import run_bass_kernel_spmd

N_CORES = 8
BATCH, IN_DIM, OUT_DIM = 4096, 4096, 8192
GPC = OUT_DIM // N_CORES          # gates per core = 1024
SLOTS = GPC // 128                # 128-gate slots per core = 8
F32 = mybir.dt.float32
F16 = mybir.dt.float16

_compiled = {}


def _build_nc():
    nc = bacc.Bacc("TRN2", target_bir_lowering=False, debug=False,
                   num_devices=N_CORES)
    aT = nc.dram_tensor("aT", [GPC, BATCH], F16, kind="ExternalInput")
    bT = nc.dram_tensor("bT", [GPC, BATCH], F16, kind="ExternalInput")
    wt = nc.dram_tensor("wt", [GPC, 16], F32, kind="ExternalInput")
    outT = nc.dram_tensor("outT", [GPC, BATCH], F16, kind="ExternalOutput")

    with tile.TileContext(nc) as tc, ExitStack() as ctx:
        const = ctx.enter_context(tc.tile_pool(name="const", bufs=1))
        pa = ctx.enter_context(tc.tile_pool(name="a", bufs=2))
        pb = ctx.enter_context(tc.tile_pool(name="b", bufs=2))
        pu = ctx.enter_context(tc.tile_pool(name="u", bufs=3))
        pw = ctx.enter_context(tc.tile_pool(name="w", bufs=3))
        po = ctx.enter_context(tc.tile_pool(name="o", bufs=2))

        # ---- per-gate coefficients from weights ----
        W = const.tile([128, SLOTS, 16], F32, tag="W")
        nc.sync.dma_start(W[:], wt.ap().rearrange("(s p) i -> p s i", p=128))
        E = const.tile([128, SLOTS, 16], F32, tag="E")
        nc.scalar.activation(E[:], W[:], mybir.ActivationFunctionType.Exp)

        def red(lo, hi, tag):
            t = const.tile([128, SLOTS], F32, tag=tag)
            nc.vector.tensor_reduce(t[:], E[:, :, lo:hi],
                                    mybir.AxisListType.X, mybir.AluOpType.add)
            return t

        Z = red(0, 16, "Z")
        R = const.tile([128, SLOTS], F32, tag="R")
        nc.vector.reciprocal(R[:], Z[:])

        # C0 = e8..e15
        C0 = red(8, 16, "C0")
        # C1 = (e2+e3) + (e6+e7) - (e8+e9) - (e12+e13)
        P23, P67, P89, P1213 = (red(2, 4, "P23"), red(6, 8, "P67"),
                                red(8, 10, "P89"), red(12, 14, "P1213"))
        C1 = const.tile([128, SLOTS], F32, tag="C1")
        nc.vector.tensor_add(C1[:], P23[:], P67[:])
        nc.vector.tensor_sub(C1[:], C1[:], P89[:])
        nc.vector.tensor_sub(C1[:], C1[:], P1213[:])
        # C2 = (e4..e7) - (e8+e9) - (e10+e11)
        P4567, P1011 = red(4, 8, "P4567"), red(10, 12, "P1011")
        C2 = const.tile([128, SLOTS], F32, tag="C2")
        nc.vector.tensor_sub(C2[:], P4567[:], P89[:])
        nc.vector.tensor_sub(C2[:], C2[:], P1011[:])
        # C3 = e1 - e2 - e4 - 2e6 - e7 + e8 + 2e9 + e11 + e13 - e14
        #    = (e1+e8+e11+e13) + 2(e9-e6) - (e2+e4+e7+e14)
        def sl(i):
            return E[:, :, i]

        C3 = const.tile([128, SLOTS], F32, tag="C3")
        t1 = const.tile([128, SLOTS], F32, tag="t1")
        nc.vector.tensor_add(C3[:], sl(1), sl(8))
        nc.vector.tensor_add(C3[:], C3[:], sl(11))
        nc.vector.tensor_add(C3[:], C3[:], sl(13))
        nc.vector.tensor_sub(t1[:], sl(9), sl(6))
        nc.vector.tensor_add(C3[:], C3[:], t1[:])
        nc.vector.tensor_add(C3[:], C3[:], t1[:])
        nc.vector.tensor_add(t1[:], sl(2), sl(4))
        nc.vector.tensor_add(t1[:], t1[:], sl(7))
        nc.vector.tensor_add(t1[:], t1[:], sl(14))
        nc.vector.tensor_sub(C3[:], C3[:], t1[:])
        # normalize by softmax denominator
        for C in (C0, C1, C2, C3):
            nc.vector.tensor_mul(C[:], C[:], R[:])

        # ---- main loop over pairs of 128-gate slots (2 MiB DMAs) ----
        for g in range(SLOTS // 2):
            s0 = g * 2
            rows = slice(s0 * 128, (s0 + 2) * 128)
            a = pa.tile([128, 2, BATCH], F16, tag="a")
            nc.sync.dma_start(
                a[:], aT.ap()[rows, :].rearrange("(j p) f -> p j f", p=128))
            b = pb.tile([128, 2, BATCH], F16, tag="b")
            nc.sync.dma_start(
                b[:], bT.ap()[rows, :].rearrange("(j p) f -> p j f", p=128))
            o = po.tile([128, 2, BATCH], F16, tag="o")
            for j in range(2):
                s = s0 + j
                aj, bj = a[:, j, :], b[:, j, :]
                # u = C3*a + C2 on ACT; w = C1*a + C0 on DVE (engine balance)
                u = pu.tile([128, BATCH], F16, tag="u")
                nc.scalar.activation(u[:], aj,
                                     mybir.ActivationFunctionType.Identity,
                                     bias=C2[:, s : s + 1],
                                     scale=C3[:, s : s + 1])
                w = pw.tile([128, BATCH], F16, tag="w")
                nc.vector.tensor_scalar(w[:], aj, C1[:, s : s + 1],
                                        C0[:, s : s + 1],
                                        mybir.AluOpType.mult,
                                        mybir.AluOpType.add)
                nc.vector.tensor_mul(u[:], u[:], bj)
                nc.vector.tensor_add(o[:, j, :], u[:], w[:])
            nc.sync.dma_start(
                outT.ap()[rows, :].rearrange("(j p) f -> p j f", p=128), o[:])

    nc.compile()
    return nc


def make_in_maps(x, weights, conn_a, conn_b):
    x = np.asarray(x, dtype=np.float32)
    weights = np.asarray(weights, dtype=np.float32)
    ca = np.asarray(conn_a).astype(np.int64)
    cb = np.asarray(conn_b).astype(np.int64)
    xT_h = np.ascontiguousarray(x.T).astype(np.float16)
    in_maps = []
    for c in range(N_CORES):
        g0, g1 = c * GPC, (c + 1) * GPC
        in_maps.append({
            "aT": np.ascontiguousarray(xT_h[ca[g0:g1]]),
            "bT": np.ascontiguousarray(xT_h[cb[g0:g1]]),
            "wt": np.ascontiguousarray(weights[g0:g1]),
        })
    return in_maps


def get_nc():
    if "nc" not in _compiled:
        _compiled["nc"] = _build_nc()
    return _compiled["nc"]


def assemble_out(results) -> np.ndarray:
    out = np.empty((BATCH, OUT_DIM), np.float32)
    for c in range(N_CORES):
        out[:, c * GPC:(c + 1) * GPC] = results[c]["outT"].T.astype(np.float32)
    return out


def kernel(x, weights, conn_a, conn_b) -> np.ndarray:
    nc = get_nc()
    in_maps = make_in_maps(x, weights, conn_a, conn_b)
    res = run_bass_kernel_spmd(nc, in_maps, core_ids=list(range(N_CORES)))
    return assemble_out(res.results)


# revision 6
# speedup vs baseline: 17.9208x; 1.2118x over previous
"""DiffLogicLayer forward on 8 TRN2 NeuronCores — v3.

Math: every one of the 16 soft logic ops is affine in {1, a, b, a*b}, so
    out[n, o] = C0[o] + C1[o]*a + C2[o]*b + C3[o]*a*b
with a = x[n, conn_a[o]], b = x[n, conn_b[o]] and C = softmax(weights) @ M
for the constant 16x4 matrix M of op coefficients.

Sharding: out_dim (gate axis) split 8 ways; each core owns 1024 gates and
the full batch. The host pre-gathers the two operand rows per gate from
x.T (pure data movement, part of the shard staging) into one packed fp16
stream abT = [a(256 rows); b(256 rows)] per pair of 128-gate slots, so the
device streams 4 MiB per pair-group with a single HWDGE DMA. Per slot:
    u = C3*a + C2   (ACT, per-partition scale/bias)
    w = C1*a + C0   (DVE tensor_scalar, per-partition scalars)
    v = u * b       (DVE, fp16 2x mode)
    o = v + w       (DVE)
Output fp16 [128, 2, 4096] per pair via the ACT-ring HWDGE (separate DGE
ring from the input loads). C0..C3 are computed on-device in f32 from the
weights shard. Host transposes/concats/casts per-core outT shards into the
full f32 output.
"""

import numpy as np
from contextlib import ExitStack

import concourse.bacc as bacc
import concourse.mybir as mybir
import concourse.tile as tile
from concourse.bass_utils import run_bass_kernel_spmd

N_CORES = 8
BATCH, IN_DIM, OUT_DIM = 4096, 4096, 8192
GPC = OUT_DIM // N_CORES          # gates per core = 1024
SLOTS = GPC // 128                # 128-gate slots per core = 8
PAIRS = SLOTS // 2                # pair-groups of 2 slots = 4
F32 = mybir.dt.float32
F16 = mybir.dt.float16

_compiled = {}


def _build_nc():
    nc = bacc.Bacc("TRN2", target_bir_lowering=False, debug=False,
                   num_devices=N_CORES)
    # abT row layout per pair g: [a slots (256 rows); b slots (256 rows)]
    abT = nc.dram_tensor("abT", [PAIRS * 512, BATCH], F16, kind="ExternalInput")
    wt = nc.dram_tensor("wt", [GPC, 16], F32, kind="ExternalInput")
    outT = nc.dram_tensor("outT", [GPC, BATCH], F16, kind="ExternalOutput")

    with tile.TileContext(nc) as tc, ExitStack() as ctx:
        const = ctx.enter_context(tc.tile_pool(name="const", bufs=1))
        pab = ctx.enter_context(tc.tile_pool(name="ab", bufs=2))
        pu = ctx.enter_context(tc.tile_pool(name="u", bufs=3))
        pw = ctx.enter_context(tc.tile_pool(name="w", bufs=3))
        po = ctx.enter_context(tc.tile_pool(name="o", bufs=2))

        # ---- per-gate coefficients from weights ----
        W = const.tile([128, SLOTS, 16], F32, tag="W")
        nc.sync.dma_start(W[:], wt.ap().rearrange("(s p) i -> p s i", p=128))
        E = const.tile([128, SLOTS, 16], F32, tag="E")
        nc.scalar.activation(E[:], W[:], mybir.ActivationFunctionType.Exp)

        def red(lo, hi, tag):
            t = const.tile([128, SLOTS], F32, tag=tag)
            nc.vector.tensor_reduce(t[:], E[:, :, lo:hi],
                                    mybir.AxisListType.X, mybir.AluOpType.add)
            return t

        Z = red(0, 16, "Z")
        R = const.tile([128, SLOTS], F32, tag="R")
        nc.vector.reciprocal(R[:], Z[:])

        # C0 = e8..e15
        C0 = red(8, 16, "C0")
        # C1 = (e2+e3) + (e6+e7) - (e8+e9) - (e12+e13)
        P23, P67, P89, P1213 = (red(2, 4, "P23"), red(6, 8, "P67"),
                                red(8, 10, "P89"), red(12, 14, "P1213"))
        C1 = const.tile([128, SLOTS], F32, tag="C1")
        nc.vector.tensor_add(C1[:], P23[:], P67[:])
        nc.vector.tensor_sub(C1[:], C1[:], P89[:])
        nc.vector.tensor_sub(C1[:], C1[:], P1213[:])
        # C2 = (e4..e7) - (e8+e9) - (e10+e11)
        P4567, P1011 = red(4, 8, "P4567"), red(10, 12, "P1011")
        C2 = const.tile([128, SLOTS], F32, tag="C2")
        nc.vector.tensor_sub(C2[:], P4567[:], P89[:])
        nc.vector.tensor_sub(C2[:], C2[:], P1011[:])
        # C3 = (e1+e8+e11+e13) + 2(e9-e6) - (e2+e4+e7+e14)
        def sl(i):
            return E[:, :, i]

        C3 = const.tile([128, SLOTS], F32, tag="C3")
        t1 = const.tile([128, SLOTS], F32, tag="t1")
        nc.vector.tensor_add(C3[:], sl(1), sl(8))
        nc.vector.tensor_add(C3[:], C3[:], sl(11))
        nc.vector.tensor_add(C3[:], C3[:], sl(13))
        nc.vector.tensor_sub(t1[:], sl(9), sl(6))
        nc.vector.tensor_add(C3[:], C3[:], t1[:])
        nc.vector.tensor_add(C3[:], C3[:], t1[:])
        nc.vector.tensor_add(t1[:], sl(2), sl(4))
        nc.vector.tensor_add(t1[:], t1[:], sl(7))
        nc.vector.tensor_add(t1[:], t1[:], sl(14))
        nc.vector.tensor_sub(C3[:], C3[:], t1[:])
        # normalize by softmax denominator
        for C in (C0, C1, C2, C3):
            nc.vector.tensor_mul(C[:], C[:], R[:])

        # ---- main loop over pair-groups (4 MiB input DMAs) ----
        for g in range(PAIRS):
            rows = slice(g * 512, (g + 1) * 512)
            ab = pab.tile([128, 4, BATCH], F16, tag="ab")
            nc.sync.dma_start(
                ab[:], abT.ap()[rows, :].rearrange("(j p) f -> p j f", p=128))
            o = po.tile([128, 2, BATCH], F16, tag="o")
            for j in range(2):
                s = g * 2 + j
                aj, bj = ab[:, j, :], ab[:, 2 + j, :]
                u = pu.tile([128, BATCH], F16, tag="u")
                nc.scalar.activation(u[:], aj,
                                     mybir.ActivationFunctionType.Identity,
                                     bias=C2[:, s : s + 1],
                                     scale=C3[:, s : s + 1])
                w = pw.tile([128, BATCH], F16, tag="w")
                nc.vector.tensor_scalar(w[:], aj, C1[:, s : s + 1],
                                        C0[:, s : s + 1],
                                        mybir.AluOpType.mult,
                                        mybir.AluOpType.add)
                nc.vector.tensor_mul(u[:], u[:], bj)
                nc.vector.tensor_add(o[:, j, :], u[:], w[:])
            nc.scalar.dma_start(
                outT.ap()[g * 256:(g + 1) * 256, :]
                    .rearrange("(j p) f -> p j f", p=128), o[:])

    nc.compile()
    return nc


def make_in_maps(x, weights, conn_a, conn_b):
    x = np.asarray(x, dtype=np.float32)
    weights = np.asarray(weights, dtype=np.float32)
    ca = np.asarray(conn_a).astype(np.int64)
    cb = np.asarray(conn_b).astype(np.int64)
    xT_h = np.ascontiguousarray(x.T).astype(np.float16)
    in_maps = []
    for c in range(N_CORES):
        g0 = c * GPC
        ab = np.empty((PAIRS * 512, BATCH), np.float16)
        for g in range(PAIRS):
            lo = g0 + g * 256
            ab[g * 512:g * 512 + 256] = xT_h[ca[lo:lo + 256]]
            ab[g * 512 + 256:(g + 1) * 512] = xT_h[cb[lo:lo + 256]]
        in_maps.append({
            "abT": ab,
            "wt": np.ascontiguousarray(weights[g0:g0 + GPC]),
        })
    return in_maps


def get_nc():
    if "nc" not in _compiled:
        _compiled["nc"] = _build_nc()
    return _compiled["nc"]


def assemble_out(results) -> np.ndarray:
    out = np.empty((BATCH, OUT_DIM), np.float32)
    for c in range(N_CORES):
        out[:, c * GPC:(c + 1) * GPC] = results[c]["outT"].T.astype(np.float32)
    return out


def kernel(x, weights, conn_a, conn_b) -> np.ndarray:
    nc = get_nc()
    in_maps = make_in_maps(x, weights, conn_a, conn_b)
    res = run_bass_kernel_spmd(nc, in_maps, core_ids=list(range(N_CORES)))
    return assemble_out(res.results)


# revision 8
# speedup vs baseline: 27.3542x; 1.5264x over previous
"""DiffLogicLayer forward on 8 TRN2 NeuronCores — v3.

Math: every one of the 16 soft logic ops is affine in {1, a, b, a*b}, so
    out[n, o] = C0[o] + C1[o]*a + C2[o]*b + C3[o]*a*b
with a = x[n, conn_a[o]], b = x[n, conn_b[o]] and C = softmax(weights) @ M
for the constant 16x4 matrix M of op coefficients.

Sharding: out_dim (gate axis) split 8 ways; each core owns 1024 gates and
the full batch. The host pre-gathers the two operand rows per gate from
x.T (pure data movement, part of the shard staging) into one packed fp16
stream abT = [a(256 rows); b(256 rows)] per pair of 128-gate slots, so the
device streams 4 MiB per pair-group with a single HWDGE DMA. Per slot:
    u = C3*a + C2   (ACT, per-partition scale/bias)
    w = C1*a + C0   (DVE tensor_scalar, per-partition scalars)
    v = u * b       (DVE, fp16 2x mode)
    o = v + w       (DVE)
Output fp16 [128, 2, 4096] per pair via the ACT-ring HWDGE (separate DGE
ring from the input loads). C0..C3 are computed on-device in f32 from the
weights shard. Host transposes/concats/casts per-core outT shards into the
full f32 output.
"""

import numpy as np
from contextlib import ExitStack

import concourse.bacc as bacc
import concourse.mybir as mybir
import concourse.tile as tile
from concourse.bass_utils import run_bass_kernel_spmd

N_CORES = 8
BATCH, IN_DIM, OUT_DIM = 4096, 4096, 8192
GPC = OUT_DIM // N_CORES          # gates per core = 1024
SLOTS = GPC // 128                # 128-gate slots per core = 8
PAIRS = SLOTS // 2                # pair-groups of 2 slots = 4
F32 = mybir.dt.float32
F16 = mybir.dt.float16

_compiled = {}


def _build_nc():
    nc = bacc.Bacc("TRN2", target_bir_lowering=False, debug=False,
                   num_devices=N_CORES)
    # abT row layout per pair g: [a slots (256 rows); b slots (256 rows)]
    abT = nc.dram_tensor("abT", [PAIRS * 512, BATCH], F16, kind="ExternalInput")
    wt = nc.dram_tensor("wt", [GPC, 16], F32, kind="ExternalInput")
    outT = nc.dram_tensor("outT", [GPC, BATCH], F16, kind="ExternalOutput")

    with tile.TileContext(nc) as tc, ExitStack() as ctx:
        const = ctx.enter_context(tc.tile_pool(name="const", bufs=1))
        pab = ctx.enter_context(tc.tile_pool(name="ab", bufs=2))
        pu = ctx.enter_context(tc.tile_pool(name="u", bufs=3))
        pw = ctx.enter_context(tc.tile_pool(name="w", bufs=3))
        po = ctx.enter_context(tc.tile_pool(name="o", bufs=2))

        # ---- per-gate coefficients from weights ----
        W = const.tile([128, SLOTS, 16], F32, tag="W")
        nc.sync.dma_start(W[:], wt.ap().rearrange("(s p) i -> p s i", p=128))
        E = const.tile([128, SLOTS, 16], F32, tag="E")
        nc.scalar.activation(E[:], W[:], mybir.ActivationFunctionType.Exp)

        def red(lo, hi, tag):
            t = const.tile([128, SLOTS], F32, tag=tag)
            nc.vector.tensor_reduce(t[:], E[:, :, lo:hi],
                                    mybir.AxisListType.X, mybir.AluOpType.add)
            return t

        Z = red(0, 16, "Z")
        R = const.tile([128, SLOTS], F32, tag="R")
        nc.vector.reciprocal(R[:], Z[:])

        # C0 = e8..e15
        C0 = red(8, 16, "C0")
        # C1 = (e2+e3) + (e6+e7) - (e8+e9) - (e12+e13)
        P23, P67, P89, P1213 = (red(2, 4, "P23"), red(6, 8, "P67"),
                                red(8, 10, "P89"), red(12, 14, "P1213"))
        C1 = const.tile([128, SLOTS], F32, tag="C1")
        nc.vector.tensor_add(C1[:], P23[:], P67[:])
        nc.vector.tensor_sub(C1[:], C1[:], P89[:])
        nc.vector.tensor_sub(C1[:], C1[:], P1213[:])
        # C2 = (e4..e7) - (e8+e9) - (e10+e11)
        P4567, P1011 = red(4, 8, "P4567"), red(10, 12, "P1011")
        C2 = const.tile([128, SLOTS], F32, tag="C2")
        nc.vector.tensor_sub(C2[:], P4567[:], P89[:])
        nc.vector.tensor_sub(C2[:], C2[:], P1011[:])
        # C3 = (e1+e8+e11+e13) + 2(e9-e6) - (e2+e4+e7+e14)
        def sl(i):
            return E[:, :, i]

        C3 = const.tile([128, SLOTS], F32, tag="C3")
        t1 = const.tile([128, SLOTS], F32, tag="t1")
        nc.vector.tensor_add(C3[:], sl(1), sl(8))
        nc.vector.tensor_add(C3[:], C3[:], sl(11))
        nc.vector.tensor_add(C3[:], C3[:], sl(13))
        nc.vector.tensor_sub(t1[:], sl(9), sl(6))
        nc.vector.tensor_add(C3[:], C3[:], t1[:])
        nc.vector.tensor_add(C3[:], C3[:], t1[:])
        nc.vector.tensor_add(t1[:], sl(2), sl(4))
        nc.vector.tensor_add(t1[:], t1[:], sl(7))
        nc.vector.tensor_add(t1[:], t1[:], sl(14))
        nc.vector.tensor_sub(C3[:], C3[:], t1[:])
        # normalize by softmax denominator
        for C in (C0, C1, C2, C3):
            nc.vector.tensor_mul(C[:], C[:], R[:])

        # ---- main loop over pair-groups (4 MiB input DMAs) ----
        for g in range(PAIRS):
            rows = slice(g * 512, (g + 1) * 512)
            ab = pab.tile([128, 4, BATCH], F16, tag="ab")
            nc.sync.dma_start(
                ab[:], abT.ap()[rows, :].rearrange("(j p) f -> p j f", p=128))
            o = po.tile([128, 2, BATCH], F16, tag="o")
            for j in range(2):
                s = g * 2 + j
                aj, bj = ab[:, j, :], ab[:, 2 + j, :]
                u = pu.tile([128, BATCH], F16, tag="u")
                nc.scalar.activation(u[:], aj,
                                     mybir.ActivationFunctionType.Identity,
                                     bias=C2[:, s : s + 1],
                                     scale=C3[:, s : s + 1])
                w = pw.tile([128, BATCH], F16, tag="w")
                nc.vector.tensor_scalar(w[:], aj, C1[:, s : s + 1],
                                        C0[:, s : s + 1],
                                        mybir.AluOpType.mult,
                                        mybir.AluOpType.add)
                nc.vector.tensor_mul(u[:], u[:], bj)
                nc.vector.tensor_add(o[:, j, :], u[:], w[:])
            nc.scalar.dma_start(
                outT.ap()[g * 256:(g + 1) * 256, :]
                    .rearrange("(j p) f -> p j f", p=128), o[:])

    nc.compile()
    return nc


def make_in_maps(x, weights, conn_a, conn_b):
    x = np.asarray(x, dtype=np.float32)
    weights = np.asarray(weights, dtype=np.float32)
    ca = np.asarray(conn_a).astype(np.int64)
    cb = np.asarray(conn_b).astype(np.int64)
    xT_h = np.ascontiguousarray(x.T).astype(np.float16)
    in_maps = []
    for c in range(N_CORES):
        g0 = c * GPC
        ab = np.empty((PAIRS * 512, BATCH), np.float16)
        for g in range(PAIRS):
            lo = g0 + g * 256
            ab[g * 512:g * 512 + 256] = xT_h[ca[lo:lo + 256]]
            ab[g * 512 + 256:(g + 1) * 512] = xT_h[cb[lo:lo + 256]]
        in_maps.append({
            "abT": ab,
            "wt": np.ascontiguousarray(weights[g0:g0 + GPC]),
        })
    return in_maps


def get_nc():
    if "nc" not in _compiled:
        _compiled["nc"] = _build_nc()
    return _compiled["nc"]


def assemble_out(results) -> np.ndarray:
    out = np.empty((BATCH, OUT_DIM), np.float32)
    for c in range(N_CORES):
        out[:, c * GPC:(c + 1) * GPC] = results[c]["outT"].T.astype(np.float32)
    return out


def kernel(x, weights, conn_a, conn_b) -> np.ndarray:
    nc = get_nc()
    in_maps = make_in_maps(x, weights, conn_a, conn_b)
    res = run_bass_kernel_spmd(nc, in_maps, core_ids=list(range(N_CORES)))
    return assemble_out(res.results)


# revision 10
# speedup vs baseline: 28.5564x; 1.0440x over previous
"""DiffLogicLayer forward on 8 TRN2 NeuronCores — v3.

Math: every one of the 16 soft logic ops is affine in {1, a, b, a*b}, so
    out[n, o] = C0[o] + C1[o]*a + C2[o]*b + C3[o]*a*b
with a = x[n, conn_a[o]], b = x[n, conn_b[o]] and C = softmax(weights) @ M
for the constant 16x4 matrix M of op coefficients.

Sharding: out_dim (gate axis) split 8 ways; each core owns 1024 gates and
the full batch. The host pre-gathers the two operand rows per gate from
x.T (pure data movement, part of the shard staging) into one packed fp16
stream abT = [a(256 rows); b(256 rows)] per pair of 128-gate slots, so the
device streams 4 MiB per pair-group with a single HWDGE DMA. Per slot:
    u = C3*a + C2   (ACT, per-partition scale/bias)
    w = C1*a + C0   (DVE tensor_scalar, per-partition scalars)
    v = u * b       (DVE, fp16 2x mode)
    o = v + w       (DVE)
Output fp16 [128, 2, 4096] per pair via the ACT-ring HWDGE (separate DGE
ring from the input loads). C0..C3 are computed on-device in f32 from the
weights shard. Host transposes/concats/casts per-core outT shards into the
full f32 output.
"""

import numpy as np
from contextlib import ExitStack

import concourse.bacc as bacc
import concourse.mybir as mybir
import concourse.tile as tile
from concourse.bass_utils import run_bass_kernel_spmd

N_CORES = 8
BATCH, IN_DIM, OUT_DIM = 4096, 4096, 8192
GPC = OUT_DIM // N_CORES          # gates per core = 1024
SLOTS = GPC // 128                # 128-gate slots per core = 8
PAIRS = SLOTS // 2                # pair-groups of 2 slots = 4
F32 = mybir.dt.float32
F16 = mybir.dt.float16

_compiled = {}


def _build_nc():
    nc = bacc.Bacc("TRN2", target_bir_lowering=False, debug=False,
                   num_devices=N_CORES)
    # abT row layout per pair g: [a slots (256 rows); b slots (256 rows)]
    abT = nc.dram_tensor("abT", [PAIRS * 512, BATCH], F16, kind="ExternalInput")
    wt = nc.dram_tensor("wt", [GPC, 16], F32, kind="ExternalInput")
    outT = nc.dram_tensor("outT", [GPC, BATCH], F16, kind="ExternalOutput")

    with tile.TileContext(nc) as tc, ExitStack() as ctx:
        const = ctx.enter_context(tc.tile_pool(name="const", bufs=1))
        pab = ctx.enter_context(tc.tile_pool(name="ab", bufs=2))
        pu = ctx.enter_context(tc.tile_pool(name="u", bufs=3))
        pw = ctx.enter_context(tc.tile_pool(name="w", bufs=3))
        po = ctx.enter_context(tc.tile_pool(name="o", bufs=2))

        # ---- per-gate coefficients from weights ----
        W = const.tile([128, SLOTS, 16], F32, tag="W")
        nc.sync.dma_start(W[:], wt.ap().rearrange("(s p) i -> p s i", p=128))
        E = const.tile([128, SLOTS, 16], F32, tag="E")
        nc.scalar.activation(E[:], W[:], mybir.ActivationFunctionType.Exp)

        def red(lo, hi, tag):
            t = const.tile([128, SLOTS], F32, tag=tag)
            nc.vector.tensor_reduce(t[:], E[:, :, lo:hi],
                                    mybir.AxisListType.X, mybir.AluOpType.add)
            return t

        Z = red(0, 16, "Z")
        R = const.tile([128, SLOTS], F32, tag="R")
        nc.vector.reciprocal(R[:], Z[:])

        # C0 = e8..e15
        C0 = red(8, 16, "C0")
        # C1 = (e2+e3) + (e6+e7) - (e8+e9) - (e12+e13)
        P23, P67, P89, P1213 = (red(2, 4, "P23"), red(6, 8, "P67"),
                                red(8, 10, "P89"), red(12, 14, "P1213"))
        C1 = const.tile([128, SLOTS], F32, tag="C1")
        nc.vector.tensor_add(C1[:], P23[:], P67[:])
        nc.vector.tensor_sub(C1[:], C1[:], P89[:])
        nc.vector.tensor_sub(C1[:], C1[:], P1213[:])
        # C2 = (e4..e7) - (e8+e9) - (e10+e11)
        P4567, P1011 = red(4, 8, "P4567"), red(10, 12, "P1011")
        C2 = const.tile([128, SLOTS], F32, tag="C2")
        nc.vector.tensor_sub(C2[:], P4567[:], P89[:])
        nc.vector.tensor_sub(C2[:], C2[:], P1011[:])
        # C3 = (e1+e8+e11+e13) + 2(e9-e6) - (e2+e4+e7+e14)
        def sl(i):
            return E[:, :, i]

        C3 = const.tile([128, SLOTS], F32, tag="C3")
        t1 = const.tile([128, SLOTS], F32, tag="t1")
        nc.vector.tensor_add(C3[:], sl(1), sl(8))
        nc.vector.tensor_add(C3[:], C3[:], sl(11))
        nc.vector.tensor_add(C3[:], C3[:], sl(13))
        nc.vector.tensor_sub(t1[:], sl(9), sl(6))
        nc.vector.tensor_add(C3[:], C3[:], t1[:])
        nc.vector.tensor_add(C3[:], C3[:], t1[:])
        nc.vector.tensor_add(t1[:], sl(2), sl(4))
        nc.vector.tensor_add(t1[:], t1[:], sl(7))
        nc.vector.tensor_add(t1[:], t1[:], sl(14))
        nc.vector.tensor_sub(C3[:], C3[:], t1[:])
        # normalize by softmax denominator
        for C in (C0, C1, C2, C3):
            nc.vector.tensor_mul(C[:], C[:], R[:])

        # ---- main loop over pair-groups (4 MiB input DMAs) ----
        for g in range(PAIRS):
            rows = slice(g * 512, (g + 1) * 512)
            ab = pab.tile([128, 4, BATCH], F16, tag="ab")
            nc.sync.dma_start(
                ab[:], abT.ap()[rows, :].rearrange("(j p) f -> p j f", p=128))
            o = po.tile([128, 2, BATCH], F16, tag="o")
            for j in range(2):
                s = g * 2 + j
                aj, bj = ab[:, j, :], ab[:, 2 + j, :]
                u = pu.tile([128, BATCH], F16, tag="u")
                nc.scalar.activation(u[:], aj,
                                     mybir.ActivationFunctionType.Identity,
                                     bias=C2[:, s : s + 1],
                                     scale=C3[:, s : s + 1])
                w = pw.tile([128, BATCH], F16, tag="w")
                nc.vector.tensor_scalar(w[:], aj, C1[:, s : s + 1],
                                        C0[:, s : s + 1],
                                        mybir.AluOpType.mult,
                                        mybir.AluOpType.add)
                nc.vector.tensor_mul(u[:], u[:], bj)
                nc.vector.tensor_add(o[:, j, :], u[:], w[:])
            nc.scalar.dma_start(
                outT.ap()[g * 256:(g + 1) * 256, :]
                    .rearrange("(j p) f -> p j f", p=128), o[:])

    nc.compile()
    return nc


def make_in_maps(x, weights, conn_a, conn_b):
    x = np.asarray(x, dtype=np.float32)
    weights = np.asarray(weights, dtype=np.float32)
    ca = np.asarray(conn_a).astype(np.int64)
    cb = np.asarray(conn_b).astype(np.int64)
    xT_h = np.ascontiguousarray(x.T).astype(np.float16)
    in_maps = []
    for c in range(N_CORES):
        g0 = c * GPC
        ab = np.empty((PAIRS * 512, BATCH), np.float16)
        for g in range(PAIRS):
            lo = g0 + g * 256
            ab[g * 512:g * 512 + 256] = xT_h[ca[lo:lo + 256]]
            ab[g * 512 + 256:(g + 1) * 512] = xT_h[cb[lo:lo + 256]]
        in_maps.append({
            "abT": ab,
            "wt": np.ascontiguousarray(weights[g0:g0 + GPC]),
        })
    return in_maps


def get_nc():
    if "nc" not in _compiled:
        _compiled["nc"] = _build_nc()
    return _compiled["nc"]


def assemble_out(results) -> np.ndarray:
    out = np.empty((BATCH, OUT_DIM), np.float32)
    for c in range(N_CORES):
        out[:, c * GPC:(c + 1) * GPC] = results[c]["outT"].T.astype(np.float32)
    return out


def kernel(x, weights, conn_a, conn_b) -> np.ndarray:
    nc = get_nc()
    in_maps = make_in_maps(x, weights, conn_a, conn_b)
    res = run_bass_kernel_spmd(nc, in_maps, core_ids=list(range(N_CORES)))
    return assemble_out(res.results)
